# revision 1
# baseline (speedup 1.0000x reference)
# Trainium2 Bass kernel for ContinuousRelativePositionalBias.
#
# Computation (see reference): for each (b, i, j) pair,
#   pos = grid_q[i] - grid_kv[b, j]                       (3 channels)
#   bias = sign(pos) * log1p(|pos|)
#   out[b, :, i, j] = W3.T @ relu(W2.T @ relu(W1.T @ bias + b1) + b2) + b3
# followed by the rearrange '(b g) i j o -> b (g o) i j' with g = 4.
#
# Sharding: the (b, i) grid is split across 8 cores: core c handles
# batch b = c // 2 and i-half ih = c % 2 (256 i values), all 512 j.
# MLP weights are replicated on every core.  Each core produces
# out[0, 4b:4b+4, ih*256:(ih+1)*256, :].
#
# Per-core pipeline (131072 points, 64 blocks of 4 i x 512 j):
#  - sign-log transform in a packed [96, 512] layout (32 i-groups of 3
#    channels stacked on partitions) so DVE/ACT run ~fully utilized;
#    log1p via one ACT Ln op (Ln(|t|+1)); sign restored with bitwise
#    ops on the fp32 bits.
#  - the packed tile bounces through DRAM so one strided readback DMA
#    per block lands it in the [3, 2048] rhs layout the PE needs
#    (SBUF APs cannot split the partition dim; DRAM APs can).
#  - L1 (K=3), L2 (K=128) matmuls in fp32r (4x the fp32 column rate),
#    relu1 on DVE (fused +b1), relu2 on ACT (fused +b2).
#  - L3: 4 concurrent col-tiled matmuls (tile_position=(0, 32q)) with
#    W3 zero-padded to [128, 32] so the psum bank is fully written.
#  - copyout on ACT (Identity + b3) into a 4-block staging tile; 4
#    output DMAs per 4 blocks.

import numpy as np
from contextlib import ExitStack

import concourse.bass as bass
import concourse.tile as tile
from concourse import mybir
from concourse.bass_utils import run_bass_kernel_spmd
from concourse.vector_clock import ScopedClock

F32 = mybir.dt.float32
F32R = mybir.dt.float32r
U32 = mybir.dt.uint32
AF = mybir.ActivationFunctionType
ALU = mybir.AluOpType

N_CORES = 8
B, I, J = 4, 512, 512
I_CORE = 256  # i values per core
N_TILES = 8  # transform tiles per core, 32 i each
G_PER_TILE = 32
BLKS_PER_TILE = 8  # blocks per transform tile, 4 i-groups each

# fp32r runs the PE at 1 column/cycle (vs 4 for fp32) at a small
# precision cost (bf16-pair decomposition in hardware).
MM_DT = F32


def _mm_ap(ap):
    return ap.bitcast(MM_DT) if MM_DT != F32 else ap


class _TileContext(tile.TileContext):
    """TileContext whose final drain splits its semaphore waits.

    The walrus build in this container rejects a Drain instruction with
    more than one sync wait; the stock TileContext exit emits one drain
    waiting on every live semaphore.  Split it into a chain of drains,
    one wait each, which is semantically identical.
    """

    MAX_WAITS = 1

    def _split_excess_waits(self):
        """Walrus in this container accepts very few sync waits per
        instruction (a Drain tolerates exactly one).  Move excess waits
        onto single-wait NOPs inserted immediately before the owning
        instruction in its basic block (same engine => executes in order
        right before it; semantically identical)."""
        nc = self.nc
        sync_info_cls = None
        for f in nc.m.functions:
            for bb in f.blocks:
                insts = bb.instructions
                out = []
                changed = False
                for inst in insts:
                    si = inst.sync_info
                    if si is not None and si.on_wait and len(si.on_wait) > self.MAX_WAITS:
                        waits = list(si.on_wait)
                        if sync_info_cls is None:
                            sync_info_cls = type(si)
                        for w in waits[: -self.MAX_WAITS]:
                            nop = mybir.InstNoOp(
                                name=nc.get_next_instruction_name()
                            )
                            nop.engine = inst.engine
                            nop.sync_info = sync_info_cls(
                                on_wait=[w], on_update=[]
                            )
                            nc.register_instruction(nop, overwrite=True)
                            out.append(nop)
                        si.on_wait = waits[-self.MAX_WAITS :]
                        changed = True
                    out.append(inst)
                if changed:
                    bb.instructions = out

    def _drain_and_barrier(self, tick_clock, wait_clock):
        drain_inst = self.nc.sync.drain()
        wait_clock.add_sem_waits(
            drain_inst.ins, ScopedClock({None: tick_clock.global_clock})
        )
        si = drain_inst.ins.sync_info
        if si is not None and si.on_wait and len(si.on_wait) > 1:
            waits = list(si.on_wait)
            si.on_wait = waits[:1]
            sync_info_cls = type(si)
            for w in waits[1:]:
                d2 = self.nc.sync.drain()
                d2.ins.sync_info = sync_info_cls(on_wait=[w], on_update=[])
        self.nc.all_engine_barrier()
        assert self.sems is not None
        popped = self.nc._tile_sem_poison_stack.pop()
        assert popped is self._sem_poison
        self.nc.clear_and_free_semaphores(list(self.sems.allocated().values()))
        self.nc.all_engine_barrier()
        self._split_excess_waits()


def build_bass():
    nc = bass.Bass("TRN2", target_bir_lowering=False, debug=False)

    kv_rep = nc.dram_tensor("kv_rep", [96, 512], F32, kind="ExternalInput").ap()
    gq_pack = nc.dram_tensor("gq_pack", [96, N_TILES], F32, kind="ExternalInput").ap()
    w1n = nc.dram_tensor("w1n", [3, 128], F32, kind="ExternalInput").ap()
    w2 = nc.dram_tensor("w2", [128, 128], F32, kind="ExternalInput").ap()
    w3p = nc.dram_tensor("w3p", [128, 32], F32, kind="ExternalInput").ap()
    b1c = nc.dram_tensor("b1c", [128, 1], F32, kind="ExternalInput").ap()
    b2c = nc.dram_tensor("b2c", [128, 1], F32, kind="ExternalInput").ap()
    b3r = nc.dram_tensor("b3r", [128, 1], F32, kind="ExternalInput").ap()
    out = nc.dram_tensor("out", [4, I_CORE, 512], F32, kind="ExternalOutput").ap()

    with _TileContext(nc) as tc, ExitStack() as ctx:
        const = ctx.enter_context(tc.tile_pool(name="const", bufs=1))
        tf = ctx.enter_context(tc.tile_pool(name="tf", bufs=2))
        yp = ctx.enter_context(tc.tile_pool(name="yp", bufs=2))
        dstage = ctx.enter_context(tc.tile_pool(name="dstage", bufs=2, space="DRAM"))
        rp = ctx.enter_context(tc.tile_pool(name="rp", bufs=3))
        h1p = ctx.enter_context(tc.tile_pool(name="h1p", bufs=3))
        h2p = ctx.enter_context(tc.tile_pool(name="h2p", bufs=3))
        sop = ctx.enter_context(tc.tile_pool(name="sop", bufs=2))
        # PSUM: 8 banks, no slot sharing, so every pool ring couples only
        # producer->consumer pairs of the same stage:
        #   pp1: [128, 512] x3 slots (3 banks) - L1 out, drained by relu1
        #   pp2: [128, 1024] x2 slots (4 banks) - L2 out, drained by relu2
        #   pp3: [128, 512] x1 slot (1 bank) - L3 out (4 col strips over
        #        two consecutive blocks), drained by copyout
        pp1 = ctx.enter_context(tc.tile_pool(name="pp1", bufs=3, space="PSUM"))
        pp2 = ctx.enter_context(tc.tile_pool(name="pp2", bufs=2, space="PSUM"))
        pp3 = ctx.enter_context(tc.tile_pool(name="pp3", bufs=1, space="PSUM"))

        kv_t = const.tile([96, 512], F32)
        nc.sync.dma_start(kv_t[:], kv_rep)
        gq_t = const.tile([96, N_TILES], F32)
        nc.sync.dma_start(gq_t[:], gq_pack)
        w1_t = const.tile([3, 128], F32)
        nc.sync.dma_start(w1_t[:], w1n)
        w2_t = const.tile([128, 128], F32)
        nc.sync.dma_start(w2_t[:], w2)
        w3_t = const.tile([128, 32], F32)
        nc.sync.dma_start(w3_t[:], w3p)
        b1_t = const.tile([128, 1], F32)
        nc.sync.dma_start(b1_t[:], b1c)
        b2_t = const.tile([128, 1], F32)
        nc.sync.dma_start(b2_t[:], b2c)
        b3_t = const.tile([128, 1], F32)
        nc.sync.dma_start(b3_t[:], b3r)

        n_blocks = 128  # 1024 points (2 i-groups) each
        h2_tiles = {}
        p3_tiles = {}
        so_tiles = {}
        r_tiles = {}
        stg_cur = None

        # Software pipeline: iteration n runs block n (2 i-groups x 512 j)
        # through L1/relu1/L2/relu2, and block n-1 through L3/copyout.  The
        # one-block skew keeps the PE from stalling on relu2 before L3.
        # Pairs of blocks share one readback, one L3 psum bank (4 col
        # strips) and one copyout.
        for n in range(n_blocks + 1):
            if n < n_blocks:
                if n % 16 == 0:
                    t = n // 16
                    # tt = gkv - gq (= -pos); the sign flip is folded into -W1.
                    tt = tf.tile([96, 512], F32, tag="tt")
                    nc.vector.tensor_scalar(
                        tt[:], kv_t[:], gq_t[:, t : t + 1], None, op0=ALU.subtract
                    )
                    aa = tf.tile([96, 512], F32, tag="aa")
                    nc.vector.tensor_scalar(
                        aa[:].bitcast(U32), tt[:].bitcast(U32), 0x7FFFFFFF, None,
                        op0=ALU.bitwise_and,
                    )
                    ll = tf.tile([96, 512], F32, tag="ll")
                    nc.scalar.activation(ll[:], aa[:], AF.Ln, bias=1.0)
                    ss = tf.tile([96, 512], F32, tag="ss")
                    nc.vector.tensor_scalar(
                        ss[:].bitcast(U32), tt[:].bitcast(U32), 0x80000000, None,
                        op0=ALU.bitwise_and,
                    )
                    yy = yp.tile([96, 512], F32, tag="yy")
                    nc.vector.tensor_tensor(
                        yy[:].bitcast(U32), ll[:].bitcast(U32), ss[:].bitcast(U32),
                        op=ALU.bitwise_or,
                    )
                    # Bounce through DRAM so the per-pair readback can split
                    # the (group, channel) partition dim with a strided AP.
                    stg_cur = dstage.tile([96, 512], F32, tag="stg")
                    nc.sync.dma_start(stg_cur[:], yy[:])

                if n % 2 == 0:
                    # Readback for pair n//2: 4 i-groups -> rhs [3, 2048].
                    pit = (n // 2) % 8  # pair index within the transform tile
                    r = rp.tile([3, 2048], F32, tag="r")
                    r_tiles[n // 2] = r
                    src = stg_cur[12 * pit : 12 * (pit + 1), :].rearrange(
                        "(g c) j -> c g j", c=3
                    )
                    dst = r[:].rearrange("c (g j) -> c g j", g=4)
                    nc.sync.dma_start(dst, src)
                r = r_tiles[n // 2]
                roff = 1024 * (n % 2)

                # L1 (K=3) into two [128, 512] psum quarters; relu1 on DVE.
                h1 = h1p.tile([128, 1024], F32, tag="h1")
                for u in range(2):
                    p1 = pp1.tile([128, 512], F32, tag="p1", name="p1")
                    nc.tensor.matmul(
                        p1[:],
                        _mm_ap(w1_t[:]),
                        _mm_ap(r[:, roff + 512 * u : roff + 512 * (u + 1)]),
                        start=True, stop=True,
                    )
                    nc.vector.tensor_scalar(
                        h1[:, 512 * u : 512 * (u + 1)], p1[:], b1_t[:], 0.0,
                        op0=ALU.add, op1=ALU.max,
                    )

                # L3 for block n-1 (emitted here so the PE has work while
                # relu1[n] runs); strips 2*(m%2)+s of the pair's p3 bank.
                if n >= 1:
                    m = n - 1
                    h2m = h2_tiles.pop(m)
                    if m % 2 == 0:
                        p3_tiles[m // 2] = pp3.tile(
                            [128, 512], F32, tag="p3", name="p3"
                        )
                    p3 = p3_tiles[m // 2]
                    for s in range(2):
                        q = 2 * (m % 2) + s
                        nc.tensor.matmul(
                            p3[32 * q : 32 * q + 32, :],
                            _mm_ap(w3_t[:]),
                            _mm_ap(h2m[:, 512 * s : 512 * (s + 1)]),
                            start=True, stop=True, tile_position=(0, 32 * q),
                        )

                # L2 (K=128); relu2 on ACT (+b2).
                h2 = h2p.tile([128, 1024], F32, tag="h2")
                h2_tiles[n] = h2
                p2 = pp2.tile([128, 1024], F32, tag="p2", name="p2")
                for u in range(2):
                    nc.tensor.matmul(
                        p2[:, 512 * u : 512 * (u + 1)],
                        _mm_ap(w2_t[:]),
                        _mm_ap(h1[:, 512 * u : 512 * (u + 1)]),
                        start=True, stop=True,
                    )
                nc.scalar.activation(h2[:], p2[:], AF.Relu, bias=b2_t[:])

                # copyout of the completed pair (blocks m-1, m with m = n-1).
                if n >= 2 and n % 2 == 0:
                    pm = (n - 1) // 2  # == n//2 - 1
                    self_so = pm // 4
                    if pm % 4 == 0:
                        so_tiles[self_so] = sop.tile(
                            [128, 2048], F32, tag="so", name="so"
                        )
                    so = so_tiles[self_so]
                    p3 = p3_tiles.pop(pm)
                    nc.scalar.activation(
                        so[:, 512 * (pm % 4) : 512 * (pm % 4 + 1)], p3[:],
                        AF.Identity, bias=b3_t[:],
                    )
                    if pm % 4 == 3:
                        # 4 output DMAs move 16 i-rows; i = ibase + 4*mm + q.
                        ibase = 4 * (pm - 3)
                        dstv = out[:, ibase : ibase + 16, :].rearrange(
                            "o (mm q) j -> q o mm j", q=4
                        )
                        for q in range(4):
                            srcq = so[32 * q : 32 * q + 4, :].rearrange(
                                "o (mm j) -> o mm j", mm=4
                            )
                            nc.sync.dma_start(dstv[q], srcq)
            else:
                # epilogue: finish block n-1 = 127 and the last pair.
                m = n - 1
                h2m = h2_tiles.pop(m)
                p3 = p3_tiles[m // 2]
                for s in range(2):
                    q = 2 * (m % 2) + s
                    nc.tensor.matmul(
                        p3[32 * q : 32 * q + 32, :],
                        _mm_ap(w3_t[:]),
                        _mm_ap(h2m[:, 512 * s : 512 * (s + 1)]),
                        start=True, stop=True, tile_position=(0, 32 * q),
                    )
                pm = m // 2
                so = so_tiles[pm // 4]
                p3 = p3_tiles.pop(pm)
                nc.scalar.activation(
                    so[:, 512 * (pm % 4) : 512 * (pm % 4 + 1)], p3[:],
                    AF.Identity, bias=b3_t[:],
                )
                ibase = 4 * (pm - 3)
                dstv = out[:, ibase : ibase + 16, :].rearrange(
                    "o (mm q) j -> q o mm j", q=4
                )
                for q in range(4):
                    srcq = so[32 * q : 32 * q + 4, :].rearrange(
                        "o (mm j) -> o mm j", mm=4
                    )
                    nc.sync.dma_start(dstv[q], srcq)

    return nc


def make_in_maps(grid_q, grid_kv, W1, b1, W2, b2, W3, b3):
    grid_q = np.asarray(grid_q, np.float32)
    grid_kv = np.asarray(grid_kv, np.float32)
    W1 = np.asarray(W1, np.float32)
    W2 = np.asarray(W2, np.float32)
    W3 = np.asarray(W3, np.float32)
    b1 = np.asarray(b1, np.float32)
    b2 = np.asarray(b2, np.float32)
    b3 = np.asarray(b3, np.float32)

    w3p = np.zeros((128, 32), np.float32)
    w3p[:, :4] = W3
    b3r = np.zeros((128, 1), np.float32)
    for q in range(4):
        b3r[32 * q : 32 * q + 4, 0] = b3

    in_maps = []
    for c in range(N_CORES):
        b = c // 2
        ih = c % 2
        kv_rep = np.tile(grid_kv[b].T, (G_PER_TILE, 1)).astype(np.float32)  # [96, 512]
        gq_sl = grid_q[ih * I_CORE : (ih + 1) * I_CORE]  # [256, 3]
        gq_pack = (
            np.transpose(gq_sl.reshape(N_TILES, G_PER_TILE, 3), (1, 2, 0))
            .reshape(96, N_TILES)
            .astype(np.float32)
        )
        in_maps.append(
            {
                "kv_rep": np.ascontiguousarray(kv_rep),
                "gq_pack": np.ascontiguousarray(gq_pack),
                "w1n": np.ascontiguousarray(-W1),
                "w2": np.ascontiguousarray(W2),
                "w3p": w3p,
                "b1c": b1.reshape(128, 1).copy(),
                "b2c": b2.reshape(128, 1).copy(),
                "b3r": b3r,
            }
        )
    return in_maps


def assemble(results):
    out = np.empty((1, 16, I, J), np.float32)
    for c in range(N_CORES):
        b = c // 2
        ih = c % 2
        res = results[c]["out"]  # [4, 256, 512]
        out[0, 4 * b : 4 * b + 4, ih * I_CORE : (ih + 1) * I_CORE, :] = res
    return out


_NC_CACHE = {}


def run(inputs, trace=False, **kwargs):
    if "nc" not in _NC_CACHE:
        _NC_CACHE["nc"] = build_bass()
    nc = _NC_CACHE["nc"]
    in_maps = make_in_maps(**inputs)
    res = run_bass_kernel_spmd(
        nc, in_maps, core_ids=list(range(N_CORES)), trace=trace, **kwargs
    )
    return assemble(res.results), res


def kernel(**inputs):
    out, _ = run(inputs, trace=False)
    return out



# revision 16
# speedup vs baseline: 2.8609x; 2.8609x over previous
# Trainium2 Bass kernel for ContinuousRelativePositionalBias.
#
# Computation (see reference): for each (b, i, j) pair,
#   pos = grid_q[i] - grid_kv[b, j]                       (3 channels)
#   bias = sign(pos) * log1p(|pos|)
#   out[b, :, i, j] = W3.T @ relu(W2.T @ relu(W1.T @ bias + b1) + b2) + b3
# followed by the rearrange '(b g) i j o -> b (g o) i j' with g = 4.
#
# Sharding: the (b, i) grid is split across 8 cores: core c handles
# batch b = c // 2 and i-half ih = c % 2 (256 i values), all 512 j.
# MLP weights are replicated on every core.  Each core produces
# out[0, 4b:4b+4, ih*256:(ih+1)*256, :].
#
# Per-core pipeline (131072 points, 128 blocks of 2 i x 512 j):
#  - all matmuls in fp32r (1 PE column/cycle vs 4 for fp32).  The BIR
#    verifier requires every producer of an fp32r matmul input to be
#    typed fp32r, so weights / rhs / activations are F32R end-to-end
#    (np side is plain float32; the PE rounds to bf16-pairs itself).
#  - sign-log transform in a packed [96, 512] layout (32 i x 3 ch on
#    partitions): tt = gkv - gq (DVE), aa = |tt| (DVE), ll = Ln(aa+1)
#    (ACT), sg = Sign(tt) (ACT), yy = ll*sg -> f32r (DVE).  The -W1
#    weight flips the gq-gkv sign back (sign-log is odd).
#  - yy bounces through DRAM so a strided readback lands [3, 4096]
#    rhs tiles (4 blocks each) for L1 (DRAM APs can split the
#    partition dim, SBUF APs cannot).
#  - L1 (K=3) and L2 (K=128) produce [128, 512] psum halves; relu1
#    and relu2 are [128, 512] ops distributed over ACT/DVE/Pool by a
#    greedy least-loaded scheduler (the three engines together are
#    faster than the PE stream).
#  - L3 is *flipped*: out = h2.T @ W3 per 128-point chunk, producing
#    [128 pts, 4 outs] psum strips at 4 cycles each (vs 512 for the
#    direct orientation - matmul time scales with output free size).
#    Strips land in a [128, 512] supertile bank laid out so that four
#    PE transposes + one copy later yield (o, i) on partitions and j
#    on columns - the exact DMA-friendly output layout.  b3 is added
#    by the post-transpose copy as a per-partition bias.
#  - per 16-block supertile: copyout1 (psum->sbuf), 4 PE transposes
#    (fp32r, 1.5 cyc/row), copyout2 (+b3), 4 output DMAs (one per o).

import numpy as np
from contextlib import ExitStack

import concourse.bass as bass
import concourse.tile as tile
from concourse import mybir
from concourse.bass_utils import run_bass_kernel_spmd
from concourse.vector_clock import ScopedClock

F32 = mybir.dt.float32
F32R = mybir.dt.float32r
U32 = mybir.dt.uint32
AF = mybir.ActivationFunctionType
ALU = mybir.AluOpType

N_CORES = 8
B, I, J = 4, 512, 512
I_CORE = 256  # i values per core
N_TILES = 8  # transform tiles per core, 32 i each
G_PER_TILE = 32
N_BLOCKS = 256  # 1 i x 512 j each


class _TileContext(tile.TileContext):
    """TileContext whose final drain splits its semaphore waits.

    The walrus build in this container rejects a Drain instruction with
    more than one sync wait; the stock TileContext exit emits one drain
    waiting on every live semaphore.  Split it into a chain of drains,
    one wait each, which is semantically identical.
    """

    MAX_WAITS = 1

    def _split_excess_waits(self):
        """Walrus in this container accepts very few sync waits per
        instruction (a Drain tolerates exactly one).  Move excess waits
        onto single-wait NOPs inserted immediately before the owning
        instruction in its basic block (same engine => executes in order
        right before it; semantically identical)."""
        nc = self.nc
        sync_info_cls = None
        for f in nc.m.functions:
            for bb in f.blocks:
                insts = bb.instructions
                out = []
                changed = False
                for inst in insts:
                    si = inst.sync_info
                    if si is not None and si.on_wait and len(si.on_wait) > self.MAX_WAITS:
                        waits = list(si.on_wait)
                        if sync_info_cls is None:
                            sync_info_cls = type(si)
                        for w in waits[: -self.MAX_WAITS]:
                            nop = mybir.InstNoOp(
                                name=nc.get_next_instruction_name()
                            )
                            nop.engine = inst.engine
                            nop.sync_info = sync_info_cls(
                                on_wait=[w], on_update=[]
                            )
                            nc.register_instruction(nop, overwrite=True)
                            out.append(nop)
                        si.on_wait = waits[-self.MAX_WAITS :]
                        changed = True
                    out.append(inst)
                if changed:
                    bb.instructions = out

    def _drain_and_barrier(self, tick_clock, wait_clock):
        drain_inst = self.nc.sync.drain()
        wait_clock.add_sem_waits(
            drain_inst.ins, ScopedClock({None: tick_clock.global_clock})
        )
        si = drain_inst.ins.sync_info
        if si is not None and si.on_wait and len(si.on_wait) > 1:
            waits = list(si.on_wait)
            si.on_wait = waits[:1]
            sync_info_cls = type(si)
            for w in waits[1:]:
                d2 = self.nc.sync.drain()
                d2.ins.sync_info = sync_info_cls(on_wait=[w], on_update=[])
        self.nc.all_engine_barrier()
        assert self.sems is not None
        popped = self.nc._tile_sem_poison_stack.pop()
        assert popped is self._sem_poison
        self.nc.clear_and_free_semaphores(list(self.sems.allocated().values()))
        self.nc.all_engine_barrier()
        self._split_excess_waits()


class _Rotation:
    """Greedy least-loaded assignment of [128, 512] vector ops across
    ACT / DVE / Pool, using cost-model per-op estimates (ns)."""

    # Pool (GPSIMD) cannot access PSUM, so relu/copyout (psum-sourced)
    # run on ACT/DVE only; Pool absorbs the SBUF->SBUF transform ops.
    COST = {
        "relu": {"A": 604.0, "D": 658.0},
        "copy": {"A": 604.0, "D": 658.0},
    }

    def __init__(self, nc):
        self.nc = nc
        self.load = {"A": 0.0, "D": 0.0, "P": 0.0}

    def fixed(self, eng, ns):
        self.load[eng] += ns

    def pick(self, kind, allow="AD"):
        costs = self.COST[kind]
        eng = min(allow, key=lambda e: self.load[e] + costs[e])
        self.load[eng] += costs[eng]
        return eng

    def relu(self, dst, src, bias, allow="AD"):
        eng = self.pick("relu", allow)
        nc = self.nc
        if eng == "A":
            nc.scalar.activation(dst, src, AF.Relu, bias=bias)
        elif eng == "D":
            nc.vector.tensor_scalar(dst, src, bias, 0.0, op0=ALU.add, op1=ALU.max)
        else:
            nc.gpsimd.tensor_scalar(dst, src, bias, 0.0, op0=ALU.add, op1=ALU.max)

    def copy(self, dst, src, bias=None, allow="AD"):
        eng = self.pick("copy", allow)
        nc = self.nc
        if eng == "A":
            nc.scalar.activation(dst, src, AF.Identity, bias=0.0 if bias is None else bias)
        elif eng == "D":
            if bias is None:
                nc.vector.tensor_copy(dst, src)
            else:
                nc.vector.tensor_scalar(dst, src, bias, None, op0=ALU.add)
        else:
            if bias is None:
                nc.gpsimd.tensor_copy(dst, src)
            else:
                nc.gpsimd.tensor_scalar(dst, src, bias, None, op0=ALU.add)


def build_bass():
    nc = bass.Bass("TRN2", target_bir_lowering=False, debug=False)

    kv_rep = nc.dram_tensor("kv_rep", [96, 512], F32, kind="ExternalInput").ap()
    gq_pack = nc.dram_tensor("gq_pack", [96, N_TILES], F32, kind="ExternalInput").ap()
    w1n = nc.dram_tensor("w1n", [3, 128], F32R, kind="ExternalInput").ap()
    w2 = nc.dram_tensor("w2", [128, 128], F32R, kind="ExternalInput").ap()
    w3 = nc.dram_tensor("w3", [128, 4], F32R, kind="ExternalInput").ap()
    ident = nc.dram_tensor("ident", [128, 128], F32R, kind="ExternalInput").ap()
    b1c = nc.dram_tensor("b1c", [128, 1], F32, kind="ExternalInput").ap()
    b2c = nc.dram_tensor("b2c", [128, 1], F32, kind="ExternalInput").ap()
    b3col = nc.dram_tensor("b3col", [128, 1], F32, kind="ExternalInput").ap()
    out = nc.dram_tensor("out", [4, I_CORE, 512], F32R, kind="ExternalOutput").ap()

    with _TileContext(nc) as tc, ExitStack() as ctx:
        const = ctx.enter_context(tc.tile_pool(name="const", bufs=1))
        tf = ctx.enter_context(tc.tile_pool(name="tf", bufs=2))
        yp = ctx.enter_context(tc.tile_pool(name="yp", bufs=2))
        dstage = ctx.enter_context(tc.tile_pool(name="dstage", bufs=2, space="DRAM"))
        rp = ctx.enter_context(tc.tile_pool(name="rp", bufs=4))
        h1p = ctx.enter_context(tc.tile_pool(name="h1p", bufs=4))
        h2p = ctx.enter_context(tc.tile_pool(name="h2p", bufs=3))
        sap = ctx.enter_context(tc.tile_pool(name="sap", bufs=2))
        sop = ctx.enter_context(tc.tile_pool(name="sop", bufs=2))
        # PSUM: 8 banks total.
        #   pp1: 3 x [128, 512] - L1 halves, drained by relu1
        #   pp2: 3 x [128, 512] - L2 halves, drained by relu2
        #   pp3: 1 x [128, 512] - flipped-L3 strips for one supertile
        #   ptp: 1 x [128, 512] - PE-transpose output
        pp1 = ctx.enter_context(tc.tile_pool(name="pp1", bufs=3, space="PSUM"))
        pp2 = ctx.enter_context(tc.tile_pool(name="pp2", bufs=3, space="PSUM"))
        pp3 = ctx.enter_context(tc.tile_pool(name="pp3", bufs=1, space="PSUM"))
        ptp = ctx.enter_context(tc.tile_pool(name="ptp", bufs=1, space="PSUM"))

        kv_t = const.tile([96, 512], F32)
        nc.sync.dma_start(kv_t[:], kv_rep)
        gq_t = const.tile([96, N_TILES], F32)
        nc.sync.dma_start(gq_t[:], gq_pack)
        w1_t = const.tile([3, 128], F32R)
        nc.sync.dma_start(w1_t[:], w1n)
        w2_t = const.tile([128, 128], F32R)
        nc.sync.dma_start(w2_t[:], w2)
        w3_t = const.tile([128, 4], F32R)
        nc.sync.dma_start(w3_t[:], w3)
        id_t = const.tile([128, 128], F32R)
        nc.sync.dma_start(id_t[:], ident)
        b1_t = const.tile([128, 1], F32)
        nc.sync.dma_start(b1_t[:], b1c)
        b2_t = const.tile([128, 1], F32)
        nc.sync.dma_start(b2_t[:], b2c)
        b3_t = const.tile([128, 1], F32)
        nc.sync.dma_start(b3_t[:], b3col)

        rot = _Rotation(nc)
        stg = {}
        r_tiles = {}
        h2_tiles = {}
        p3_cur = [None]

        tf_state = {}

        def transform_step(t, step):
            # Transform ops for tile t, spread one per iteration so the
            # [96, 512] bursts don't pile up in front of the relus.
            # tt = gkv - gq (-W1 flips the sign back); yy = sign*log1p.
            if step == 0:
                tt = tf.tile([96, 512], F32, tag="tt")
                tf_state["tt"] = tt
                nc.vector.tensor_scalar(
                    tt[:], kv_t[:], gq_t[:, t : t + 1], None, op0=ALU.subtract
                )
                rot.fixed("D", 594.0)
            elif step == 1:
                aa = tf.tile([96, 512], F32, tag="aa")
                tf_state["aa"] = aa
                nc.scalar.activation(aa[:], tf_state["tt"][:], AF.Abs)
                rot.fixed("A", 612.0)
            elif step == 2:
                ll = tf.tile([96, 512], F32, tag="ll")
                tf_state["ll"] = ll
                nc.scalar.activation(ll[:], tf_state["aa"][:], AF.Ln, bias=1.0)
                rot.fixed("A", 612.0)
            elif step == 3:
                sg = tf.tile([96, 512], F32, tag="sg")
                tf_state["sg"] = sg
                nc.scalar.activation(sg[:], tf_state["tt"][:], AF.Sign)
                rot.fixed("A", 612.0)
            elif step == 4:
                yy = yp.tile([96, 512], F32R, tag="yy")
                tf_state["yy"] = yy
                nc.gpsimd.tensor_tensor(
                    yy[:], tf_state["ll"][:], tf_state["sg"][:], op=ALU.mult
                )
            else:
                s = dstage.tile([96, 512], F32R, tag="stg")
                stg[t] = s
                nc.sync.dma_start(s[:], tf_state["yy"][:])

        def transform(t):
            for step in range(6):
                transform_step(t, step)

        def readback(k):
            # blocks (= i values) [8k, 8k+8): 8 i-groups -> rhs [3, 4096].
            t, q = k // 4, k % 4
            r = rp.tile([3, 4096], F32R, tag="r")
            r_tiles[k] = r
            src = stg[t][24 * q : 24 * (q + 1), :].rearrange(
                "(g c) j -> c g j", c=3
            )
            dst = r[:].rearrange("c (g j) -> c g j", g=8)
            nc.sync.dma_start(dst, src)

        def l3_group(m):
            # Flipped L3 for block m (one i value): 4 chunks of 128 j,
            # each out[j, o] = h2[:, chunk].T @ W3 into contiguous psum
            # cols [16t + 4cp, +4), t = m % 32.  PSUM start=True lazily
            # zeroes the whole 2 KB bank row, so only the supertile's
            # first matmul sets it; every byte is written exactly once.
            if m % 32 == 0:
                p3_cur[0] = pp3.tile([128, 512], F32, tag="p3", name="p3")
            p3 = p3_cur[0]
            h2m = h2_tiles.pop(m)
            t = m % 32
            for cp in range(4):
                base = 16 * t + 4 * cp
                nc.tensor.matmul(
                    p3[:, base : base + 4],
                    h2m[:, 128 * cp : 128 * (cp + 1)],
                    w3_t[:],
                    start=(t == 0 and cp == 0), stop=True,
                    skip_group_check=True,
                )

        def finish_supertile(g):
            # copyout1 permutes psum cols (t, cp, o) -> sbuf cols
            # 128*cp + 32*o + t so the four PE transposes land (o, t) on
            # partitions and j = 128*cp + p on columns - the output
            # layout.  copyout2 adds b3 (per partition = per o).
            p3 = p3_cur[0]
            sa = sap.tile([128, 512], F32R, tag="sa")
            src = p3[:].rearrange("p (t c o) -> p t c o", t=32, c=4)
            dst = sa[:].rearrange("p (c o t) -> p t c o", c=4, o=4)
            rot.copy(dst, src)
            pt = ptp.tile([128, 512], F32R, tag="pt", name="pt")
            for kk in range(4):
                nc.tensor.matmul(
                    pt[:, 128 * kk : 128 * (kk + 1)],
                    sa[:, 128 * kk : 128 * (kk + 1)],
                    id_t[:],
                    is_transpose=True, start=(kk == 0), stop=True,
                    skip_group_check=True,
                )
            so = sop.tile([128, 512], F32R, tag="so")
            rot.copy(so[:], pt[:], bias=b3_t[:])
            for o in range(4):
                nc.sync.dma_start(
                    out[o, 32 * g : 32 * g + 32, :], so[32 * o : 32 * o + 32, :]
                )

        # Software pipeline, 3-block skew: iteration n runs L1(n),
        # L2(n-3), L3(n-5).  Each relu gets ~3 iterations (~1.4 us) of
        # slack between production and consumption, while the 3-slot
        # psum rings give 3 blocks of overlap, so the PE never waits on
        # the vector engines in steady state.
        h1_tiles = {}
        for n in range(N_BLOCKS + 5):
            if n == 0:
                transform(0)
                transform(1)
                for k in range(3):
                    readback(k)
            else:
                # Transform for tile n//32 + 2, one sub-op per iteration
                # (two supertiles of lead so the DRAM bounce + readback
                # complete long before L1 needs the rhs tiles).
                tnext = n // 32 + 2
                if tnext < N_TILES and 16 <= n % 32 < 22:
                    transform_step(tnext, n % 32 - 16)
                if n % 8 == 0 and n // 8 + 2 < 32:
                    readback(n // 8 + 2)

            if n < N_BLOCKS:
                r = r_tiles[n // 8]
                if n % 8 == 7:
                    del r_tiles[n // 8]

                # L1 into psum; relu1 on ACT/DVE (tightest ring).
                h1 = h1p.tile([128, 512], F32R, tag="h1")
                h1_tiles[n] = h1
                p1 = pp1.tile([128, 512], F32, tag="p1", name="p1")
                nc.tensor.matmul(
                    p1[:],
                    w1_t[:],
                    r[:, 512 * (n % 8) : 512 * (n % 8 + 1)],
                    start=True, stop=True,
                )
                rot.relu(h1[:], p1[:], b1_t[:])

            if 3 <= n < N_BLOCKS + 3:
                # L2 for block n-3; relu2 may use Pool (same slack).
                m1 = n - 3
                h1 = h1_tiles.pop(m1)
                h2 = h2p.tile([128, 512], F32R, tag="h2")
                h2_tiles[m1] = h2
                p2 = pp2.tile([128, 512], F32, tag="p2", name="p2")
                nc.tensor.matmul(
                    p2[:], w2_t[:], h1[:], start=True, stop=True,
                )
                rot.relu(h2[:], p2[:], b2_t[:])

            if n >= 5:
                m = n - 5
                l3_group(m)
                if m % 32 == 31:
                    finish_supertile(m // 32)

    return nc


def make_in_maps(grid_q, grid_kv, W1, b1, W2, b2, W3, b3):
    grid_q = np.asarray(grid_q, np.float32)
    grid_kv = np.asarray(grid_kv, np.float32)
    W1 = np.asarray(W1, np.float32)
    W2 = np.asarray(W2, np.float32)
    W3 = np.asarray(W3, np.float32)
    b1 = np.asarray(b1, np.float32)
    b2 = np.asarray(b2, np.float32)
    b3 = np.asarray(b3, np.float32)

    b3col = np.repeat(b3, 32).reshape(128, 1).astype(np.float32)
    ident = np.eye(128, dtype=np.float32)

    in_maps = []
    for c in range(N_CORES):
        b = c // 2
        ih = c % 2
        kv_rep = np.tile(grid_kv[b].T, (G_PER_TILE, 1)).astype(np.float32)  # [96, 512]
        gq_sl = grid_q[ih * I_CORE : (ih + 1) * I_CORE]  # [256, 3]
        gq_pack = (
            np.transpose(gq_sl.reshape(N_TILES, G_PER_TILE, 3), (1, 2, 0))
            .reshape(96, N_TILES)
            .astype(np.float32)
        )
        in_maps.append(
            {
                "kv_rep": np.ascontiguousarray(kv_rep),
                "gq_pack": np.ascontiguousarray(gq_pack),
                "w1n": np.ascontiguousarray(-W1),
                "w2": np.ascontiguousarray(W2),
                "w3": np.ascontiguousarray(W3),
                "ident": ident,
                "b1c": b1.reshape(128, 1).copy(),
                "b2c": b2.reshape(128, 1).copy(),
                "b3col": b3col,
            }
        )
    return in_maps


def assemble(results):
    out = np.empty((1, 16, I, J), np.float32)
    for c in range(N_CORES):
        b = c // 2
        ih = c % 2
        res = results[c]["out"]  # [4, 256, 512]
        out[0, 4 * b : 4 * b + 4, ih * I_CORE : (ih + 1) * I_CORE, :] = res
    return out


_NC_CACHE = {}


def run(inputs, trace=False, **kwargs):
    if "nc" not in _NC_CACHE:
        _NC_CACHE["nc"] = build_bass()
    nc = _NC_CACHE["nc"]
    in_maps = make_in_maps(**inputs)
    res = run_bass_kernel_spmd(
        nc, in_maps, core_ids=list(range(N_CORES)), trace=trace, **kwargs
    )
    return assemble(res.results), res


def kernel(**inputs):
    out, _ = run(inputs, trace=False)
    return out
